# revision 42
# baseline (speedup 1.0000x reference)
"""Top-1 MoE FFN kernel for Trainium2 (8 NeuronCores, expert-parallel).

Problem (hardcoded shapes):
  x:  [2048, 8, 1024] f32   tokens
  Wg: [8, 1024]       f32   gate
  W1: [8, 4096, 1024] f32   expert up-proj
  b1: [8, 4096]       f32
  W2: [8, 1024, 4096] f32   expert down-proj
  b2: [8, 1024]       f32
  out = for each token: top1-expert FFN(x) * top1_prob  (exact gelu)

Strategy: gate/top-1 routing computed on host (tiny compute, and it *is*
the sharding decision); tokens dispatched to core e = their top-1 expert;
each core runs a dense 2-layer FFN over its tokens (padded to a common
capacity C) with fp32 data and float32r matmuls on the PE; host scales by
top1_prob and scatters back.

Per-core device kernel (SPMD, identical program):
  inputs : xt [1024, C] (= routed tokens, transposed), w1t [1024, 4096]
           (= W1[e].T), w2t [4096, 1024] (= W2[e].T), b1 [128, 32],
           b2 [128, 8]
  output : yt [1024, C] (= (gelu(x W1^T + b1) W2^T + b2).T, unscaled)
Layout: first matmul keeps f on PSUM partitions / tokens on the free axis
(h stored transposed, [f, tok]) so the second matmul can contract over f
without any transposes.
"""

import numpy as np
from contextlib import ExitStack

S, NB, D, F, E = 2048, 8, 1024, 4096, 8
T = S * NB          # 16384 tokens
P = 128
NT = 512            # tokens per matmul (one PSUM bank, fp32)
KD = D // P         # 8 contraction chunks over d
KF = F // P         # 32 chunks over f
GB = 2              # token blocks per group (H tile = [128, GB*NT])
NCORES = 8

MODE = "f32r"       # "f32r" | "bf16" | "f32"
LDW_OPT = True      # let walrus dedup back-to-back same-weight LDWs

_cache = {}
_ldw_patched = False


def _enable_ldw_opt():
    """Flip walrus's --enable-ldw-opt to true (dedups consecutive
    same-source LDWEIGHTS, ~107ns of PE time each)."""
    global _ldw_patched
    if _ldw_patched:
        return
    import concourse.bass_utils as bu

    orig = bu.bir_verify_and_optimise

    def patched(tmpdir, inp="bir.json", outp="file.neff", arch=None, *,
                dve_root=None):
        real_run = bu.run_command

        def run_hook(cmd, **kw):
            cmd = [
                "--enable-ldw-opt=true" if c == "--enable-ldw-opt=false" else c
                for c in cmd
            ]
            return real_run(cmd, **kw)

        bu.run_command = run_hook
        try:
            return orig(tmpdir, inp, outp, arch, dve_root=dve_root)
        finally:
            bu.run_command = real_run

    bu.bir_verify_and_optimise = patched
    _ldw_patched = True


def _build_bass(C, mode, act="gelu", opts=None):
    import concourse.bass as bass
    import concourse.mybir as mybir
    import concourse.tile as tile
    from concourse import bacc

    o = dict(psA=4, psB=4, skip_wdma=False, skip_xdma=False, skip_out=False,
             phase="AB", w2bufs=3, xbufs=KD, w1bufs=4, outbufs=3)
    o.update(opts or {})
    FH = KF // 2  # f-chunks per batched w2 load

    f32 = mybir.dt.float32
    if mode == "bf16":
        io_dt = mybir.dt.bfloat16
    elif mode == "f32r":
        io_dt = mybir.dt.float32r
    else:
        io_dt = mybir.dt.float32

    nc = bacc.Bacc(
        "TRN2",
        target_bir_lowering=False,
        debug=False,
        num_devices=NCORES,
        enable_asserts=False,
    )
    xt_d = nc.dram_tensor("xt", [D, C], io_dt, kind="ExternalInput").ap()
    w1t_d = nc.dram_tensor("w1t", [D, F], io_dt, kind="ExternalInput").ap()
    w2t_d = nc.dram_tensor("w2t", [F, D], io_dt, kind="ExternalInput").ap()
    b1_d = nc.dram_tensor("b1", [P, KF], f32, kind="ExternalInput").ap()
    b2_d = nc.dram_tensor("b2", [P, KD], f32, kind="ExternalInput").ap()
    yt_d = nc.dram_tensor("yt", [D, C], f32, kind="ExternalOutput").ap()

    xt_v = xt_d.rearrange("(k p) c -> k p c", p=P)    # [KD, 128, C]
    w1_v = w1t_d.rearrange("(k p) f -> p k f", p=P)   # [128, KD, F], part-major
    w2_v = w2t_d.rearrange("(k p) d -> k p d", p=P)   # [KF, 128, D]
    w2_pv = w2t_d.rearrange("(c p) d -> p c d", p=P)  # [128, KF, D], part-major
    yt_v = yt_d.rearrange("(k p) c -> k p c", p=P)    # [KD, 128, C]

    # token blocks: 512-wide, with an optional 256-wide tail (fp32r needs
    # moving dim >= 256 for full PE rate)
    assert C % 256 == 0 and C >= NT
    blocks = []
    rem = C
    while rem >= NT:
        blocks.append(NT)
        rem -= NT
    if rem:
        assert rem == 256
        blocks.append(256)
    # groups of blocks with total width <= GW; block offsets
    offs = [0]
    for w in blocks:
        offs.append(offs[-1] + w)
    GW = GB * NT
    groups = []
    cur, curw = [], 0
    for i, w in enumerate(blocks):
        if cur and curw + w > GW:
            groups.append(cur)
            cur, curw = [], 0
        cur.append(i)
        curw += w
    if cur:
        groups.append(cur)
    # a lone narrow tail group would be DMA-bound (full weight stream for
    # few tokens) — rebalance a 512 block from the previous group into it
    if (
        len(groups) >= 2
        and sum(blocks[i] for i in groups[-1]) <= 256
        and len(groups[-2]) > 1
    ):
        groups[-1].insert(0, groups[-2].pop())

    ident = mybir.ActivationFunctionType.Identity
    gelu = mybir.ActivationFunctionType.Gelu if act == "gelu" else ident

    with tile.TileContext(nc) as tc, ExitStack() as ctx:
        const_pool = ctx.enter_context(tc.tile_pool(name="const", bufs=1))
        x_pool = ctx.enter_context(tc.tile_pool(name="x", bufs=o["xbufs"]))
        h_pool = ctx.enter_context(tc.tile_pool(name="h", bufs=KF))
        w1_pool = ctx.enter_context(tc.tile_pool(name="w1", bufs=o["w1bufs"]))
        w2_pool = ctx.enter_context(tc.tile_pool(name="w2", bufs=o["w2bufs"]))
        out_pool = ctx.enter_context(tc.tile_pool(name="out", bufs=o["outbufs"]))
        psA = ctx.enter_context(tc.tile_pool(name="psA", bufs=o["psA"], space="PSUM"))
        psB = ctx.enter_context(tc.tile_pool(name="psB", bufs=o["psB"], space="PSUM"))

        b1_sb = const_pool.tile([P, KF], f32)
        nc.sync.dma_start(b1_sb[:], b1_d[:])
        b2_sb = const_pool.tile([P, KD], f32)
        nc.sync.dma_start(b2_sb[:], b2_d[:])

        for grp in groups:
            bws = [blocks[i] for i in grp]
            nb = len(bws)
            c0 = offs[grp[0]]
            gw = sum(bws)
            lofs = [0]
            for w in bws:
                lofs.append(lofs[-1] + w)

            # X for this token group, all d-chunks resident; per-block DMAs
            # so the first block's tiles land quickly
            x_tiles = [x_pool.tile([P, GW], io_dt, tag="x", name=f"x_{k}")
                       for k in range(KD)]
            # x loads go out on the ACT DMA queue: keeps the SP queue free
            # to issue the first w1 loads immediately at kernel start
            if o["skip_xdma"]:
                for k in range(KD):
                    nc.scalar.dma_start(x_tiles[k][:, :8], xt_v[k, :, c0 : c0 + 8])
            else:
                for b in range(nb):
                    lo, bw = lofs[b], bws[b]
                    for k in range(KD):
                        nc.scalar.dma_start(
                            x_tiles[k][:, lo : lo + bw],
                            xt_v[k, :, c0 + lo : c0 + lo + bw],
                        )

            # ---- Phase A: h[f, tok] = gelu(W1 x + b1), staged in SBUF ----
            h_tiles = []
            for f in range(KF):
                w1_t = w1_pool.tile([P, KD * P], io_dt, tag="w1")
                if o["skip_wdma"]:
                    nc.sync.dma_start(w1_t[:, :8], w1_v[:, 0, f * P : f * P + 8])
                else:
                    nc.sync.dma_start(
                        w1_t[:].rearrange("p (k j) -> p k j", k=KD),
                        w1_v[:, :, f * P : (f + 1) * P],
                    )
                h_t = h_pool.tile([P, GW], io_dt, tag="h")
                h_tiles.append(h_t)
                if "A" not in o["phase"]:
                    nc.vector.memset(h_t[:], 0.01)
                    continue
                # k outer / b inner: consecutive matmuls share the same
                # weight tile (enables walrus ldw-opt dedup)
                phs = [
                    psA.tile([P, NT], f32, tag="psA", name=f"ph_{f}_{b}")
                    for b in range(nb)
                ]
                for k in range(KD):
                    for b in range(nb):
                        lo, bw = lofs[b], bws[b]
                        nc.tensor.matmul(
                            phs[b][:, :bw],
                            lhsT=w1_t[:, k * P : (k + 1) * P],
                            rhs=x_tiles[k][:, lo : lo + bw],
                            start=(k == 0),
                            stop=(k == KD - 1),
                        )
                for b in range(nb):
                    lo, bw = lofs[b], bws[b]
                    nc.scalar.activation(
                        h_t[:, lo : lo + bw], phs[b][:, :bw], gelu,
                        bias=b1_sb[:, f : f + 1],
                    )

            # ---- Phase B: y[dd, tok] = W2 h + b2 ----
            for dd in range(KD if "B" in o["phase"] else 1):
                if "B" not in o["phase"]:
                    for b in range(nb):
                        lo, bw = lofs[b], bws[b]
                        o_t = out_pool.tile([P, NT], f32, tag="out", name=f"oo{dd}{b}")
                        nc.scalar.activation(
                            o_t[:, :bw], h_tiles[0][:, lo : lo + bw].bitcast(f32),
                            ident)
                        nc.sync.dma_start(
                            yt_v[dd, :, c0 + lo : c0 + lo + bw], o_t[:, :bw])
                    continue
                pys = [
                    psB.tile([P, NT], f32, tag="psB", name=f"py_{dd}_{b}")
                    for b in range(nb)
                ]
                for fh in range(KF // FH):
                    w2_t = w2_pool.tile([P, FH * P], io_dt, tag="w2")
                    if o["skip_wdma"]:
                        nc.sync.dma_start(
                            w2_t[:, :8], w2_v[fh * FH, :, dd * P : dd * P + 8]
                        )
                    else:
                        nc.sync.dma_start(
                            w2_t[:].rearrange("p (c j) -> p c j", c=FH),
                            w2_pv[:, fh * FH : (fh + 1) * FH,
                                  dd * P : (dd + 1) * P],
                        )
                    for fl in range(FH):
                        f = fh * FH + fl
                        for b in range(nb):
                            lo, bw = lofs[b], bws[b]
                            nc.tensor.matmul(
                                pys[b][:, :bw],
                                lhsT=w2_t[:, fl * P : (fl + 1) * P],
                                rhs=h_tiles[f][:, lo : lo + bw],
                                start=(f == 0),
                                stop=(f == KF - 1),
                            )
                for b in range(nb):
                    lo, bw = lofs[b], bws[b]
                    o_t = out_pool.tile([P, NT], f32, tag="out")
                    nc.scalar.activation(
                        o_t[:, :bw], pys[b][:, :bw], ident,
                        bias=b2_sb[:, dd : dd + 1]
                    )
                    # issue stores from the ACT DMA queue so they don't
                    # block the SP queue's weight-load triggers
                    if o["skip_out"]:
                        nc.scalar.dma_start(
                            yt_v[dd, :, c0 + lo : c0 + lo + 8], o_t[:, :8]
                        )
                    else:
                        nc.scalar.dma_start(
                            yt_v[dd, :, c0 + lo : c0 + lo + bw], o_t[:, :bw]
                        )
    nc.compile()
    return nc


def _get_bass(C, mode):
    if LDW_OPT:
        _enable_ldw_opt()
    key = (C, mode)
    if key not in _cache:
        _cache[key] = _build_bass(C, mode)
    return _cache[key]


def _route(x, Wg):
    """Top-1 routing, mirroring the reference ops (jax on default device
    if available, else numpy f64)."""
    try:
        import jax
        import jax.numpy as jnp

        scores = jnp.einsum("snd,ed->sne", jnp.asarray(x), jnp.asarray(Wg))
        prob = jax.nn.softmax(scores, axis=-1)
        idx = jnp.argmax(prob, axis=-1)
        p1 = jnp.take_along_axis(prob, idx[..., None], axis=-1)[..., 0]
        return np.asarray(idx).reshape(-1), np.asarray(p1).reshape(-1)
    except Exception:
        xf = x.reshape(-1, x.shape[-1]).astype(np.float64)
        scores = xf @ Wg.T.astype(np.float64)
        m = scores.max(-1, keepdims=True)
        p = np.exp(scores - m)
        p /= p.sum(-1, keepdims=True)
        idx = scores.argmax(-1)
        p1 = p[np.arange(len(idx)), idx]
        return idx.astype(np.int64), p1.astype(np.float32)


def _run(inputs, trace=False, trace_cores=None, mode=MODE):
    from concourse.bass_utils import run_bass_kernel_spmd

    x = np.ascontiguousarray(np.asarray(inputs["x"], dtype=np.float32))
    Wg = np.asarray(inputs["Wg"], dtype=np.float32)
    W1 = np.asarray(inputs["W1"], dtype=np.float32)
    b1 = np.asarray(inputs["b1"], dtype=np.float32)
    W2 = np.asarray(inputs["W2"], dtype=np.float32)
    b2 = np.asarray(inputs["b2"], dtype=np.float32)

    idx, p1 = _route(x, Wg)
    xf = x.reshape(T, D)

    order = np.argsort(idx, kind="stable")
    counts = np.bincount(idx, minlength=E)
    bounds = np.concatenate([[0], np.cumsum(counts)])
    C = max(NT, int(-(-int(counts.max()) // 256)) * 256)

    if mode == "bf16":
        import ml_dtypes

        io_np = ml_dtypes.bfloat16
    else:
        io_np = np.float32

    nc = _get_bass(C, mode)

    in_maps = []
    sels = []
    for e in range(E):
        sel = order[bounds[e] : bounds[e + 1]]
        sels.append(sel)
        xt = np.zeros((D, C), dtype=io_np)
        xt[:, : len(sel)] = xf[sel].T
        in_maps.append(
            {
                "xt": xt,
                "w1t": np.ascontiguousarray(W1[e].T).astype(io_np),
                "w2t": np.ascontiguousarray(W2[e].T).astype(io_np),
                "b1": np.ascontiguousarray(b1[e].reshape(KF, P).T),
                "b2": np.ascontiguousarray(b2[e].reshape(KD, P).T),
            }
        )

    br = run_bass_kernel_spmd(
        nc,
        in_maps,
        core_ids=list(range(NCORES)),
        trace=trace,
        trace_cores=trace_cores,
    )

    yf = np.zeros((T, D), dtype=np.float32)
    for e in range(E):
        sel = sels[e]
        ye = br.results[e]["yt"][:, : len(sel)].T
        yf[sel] = ye * p1[sel, None].astype(np.float32)
    return yf.reshape(S, NB, D), br


def kernel(**inputs):
    y, _ = _run(inputs, trace=False)
    return y
